# revision 4
# baseline (speedup 1.0000x reference)
"""Causal attention kernel for Trainium2 (Bass/Tile), 8-core SPMD.

Problem: B=2, H=16, S=2048, D=64, fp32, causal mask.
Sharding: 32 (b,h) heads split 4-per-core across 8 NeuronCores.

Per-head algorithm (all fp32):
  - Q^T, K^T [64, S] built on-chip via PE transposes of natural [128, 64] blocks.
  - Scores computed transposed: S^T[k, q] = (K_blk)(Q^T) via matmul with
    lhsT = K^T block (stationary), rhs = Q^T (moving).  PSUM [128, q-chunk].
  - P^T = exp(S^T / 8) on ScalarE (PSUM->SBUF, scale fused into activation).
  - Causal mask on the diagonal 128x128 block via gpsimd.affine_select (zero fill).
  - O^T[d, q] (+ row of softmax sums) accumulated over k-blocks:
    lhsT = [V_blk | ones] [128, 65] (stationary), rhs = P^T (moving).
  - O^T transposed back per 128-q block via PE, rows scaled by 1/sums, DMA out.

No max-subtraction is needed: scores ~ N(0,1), exp stays in fp32 range.
"""

import numpy as np

B, H, S, D = 2, 16, 2048, 64
NCORES = 8
HPC = (B * H) // NCORES  # heads per core = 4
PB = 128                 # partition block
NB = S // PB             # 16 seq blocks per head
CHUNK = 1024             # q-chunk width (2 PSUM banks)
NCHUNK = S // CHUNK

_PROGRAM = None


def _build_program():
    import concourse.bacc as bacc
    import concourse.mybir as mybir
    import concourse.tile as tile
    from concourse.masks import make_identity

    FP32 = mybir.dt.float32
    nc = bacc.Bacc("TRN2", target_bir_lowering=False, debug=False,
                   enable_asserts=False)
    q = nc.dram_tensor("q", [HPC, S, D], FP32, kind="ExternalInput").ap()
    k = nc.dram_tensor("k", [HPC, S, D], FP32, kind="ExternalInput").ap()
    v = nc.dram_tensor("v", [HPC, S, D], FP32, kind="ExternalInput").ap()
    o = nc.dram_tensor("o", [HPC, S, D], FP32, kind="ExternalOutput").ap()

    with tile.TileContext(nc) as tc:
        with (
            tc.tile_pool(name="const", bufs=1) as cpool,
            tc.tile_pool(name="qk", bufs=2) as qkpool,
            tc.tile_pool(name="vp", bufs=2) as vpool,
            tc.tile_pool(name="inp", bufs=2) as inpool,
            tc.tile_pool(name="pp", bufs=3) as ppool,
            tc.tile_pool(name="onorm", bufs=2) as opool,
            tc.tile_pool(name="ps_s", bufs=2, space="PSUM") as ps_s,
            tc.tile_pool(name="ps_o", bufs=1, space="PSUM") as ps_o,
            tc.tile_pool(name="ps_t", bufs=2, space="PSUM") as ps_t,
        ):
            ident = cpool.tile([PB, PB], FP32)
            make_identity(nc, ident)

            for h in range(HPC):
                # ---- load inputs for this head (seq-block layout) ----
                qh = inpool.tile([PB, NB, D], FP32, tag="qh")
                kh = inpool.tile([PB, NB, D], FP32, tag="kh")
                nc.sync.dma_start(out=qh, in_=q[h].rearrange("(n p) d -> p n d", p=PB))
                nc.sync.dma_start(out=kh, in_=k[h].rearrange("(n p) d -> p n d", p=PB))
                vh = vpool.tile([PB, NB, D + 1], FP32, tag="vh")
                nc.sync.dma_start(out=vh[:, :, 0:D],
                                  in_=v[h].rearrange("(n p) d -> p n d", p=PB))
                nc.vector.memset(vh[:, :, D:D + 1], 1.0)

                # ---- build Q^T, K^T [64, S] via PE transposes ----
                qT = qkpool.tile([D, S], FP32, tag="qT")
                kT = qkpool.tile([D, S], FP32, tag="kT")
                for src, dstT in ((qh, qT), (kh, kT)):
                    for g in range(NB // 4):
                        stg = ps_t.tile([D, 4 * PB], FP32, tag="stg")
                        for j in range(4):
                            nc.tensor.transpose(
                                stg[:, PB * j:PB * (j + 1)],
                                src[:, 4 * g + j, :], ident)
                        nc.vector.tensor_copy(
                            dstT[:, 4 * PB * g:4 * PB * (g + 1)], stg)

                # ---- main attention loop over q-chunks ----
                for c in range(NCHUNK):
                    q0 = c * CHUNK
                    q1 = q0 + CHUNK
                    oT = ps_o.tile([D + 1, CHUNK], FP32, tag="oT")
                    jk_hi = q1 // PB - 1
                    for jk in range(jk_hi + 1):
                        lo = max(q0, PB * jk)   # causal column start (global q)
                        W = q1 - lo
                        sT = ps_s.tile([PB, CHUNK], FP32, tag="sT")
                        x = 0
                        while x < W:
                            w = min(512, W - x)
                            nc.tensor.matmul(
                                sT[:, x:x + w],
                                lhsT=kT[:, PB * jk:PB * (jk + 1)],
                                rhs=qT[:, lo + x:lo + x + w],
                                start=True, stop=True)
                            x += w
                        pT = ppool.tile([PB, CHUNK], FP32, tag="pT")
                        nc.scalar.activation(
                            pT[:, 0:W], sT[:, 0:W],
                            mybir.ActivationFunctionType.Exp,
                            scale=float(1.0 / np.sqrt(D)))
                        if PB * jk >= q0:
                            # diagonal block sits at columns [0, 128): keep q>=k
                            nc.gpsimd.affine_select(
                                out=pT[:, 0:PB], in_=pT[:, 0:PB],
                                compare_op=mybir.AluOpType.is_ge,
                                fill=0.0, base=0, channel_multiplier=-1,
                                pattern=[[1, PB]])
                        # PV accumulate, pieces aligned to oT bank boundaries
                        ostart = lo - q0
                        x = 0
                        while x < W:
                            ocol = ostart + x
                            w = min(512 - (ocol % 512), W - x)
                            nc.tensor.matmul(
                                oT[:, ocol:ocol + w],
                                lhsT=vh[:, jk, :],
                                rhs=pT[:, x:x + w],
                                start=(jk == 0), stop=(jk == jk_hi),
                                skip_group_check=True)
                            x += w

                    # ---- normalize chunk and write out ----
                    oTs = opool.tile([D + 1, CHUNK], FP32, tag="oTs")
                    nc.vector.tensor_copy(oTs, oT)
                    obuf = opool.tile([PB, CHUNK // PB, D], FP32, tag="obuf")
                    for b in range(CHUNK // PB):
                        stg2 = ps_t.tile([PB, D + 1], FP32, tag="stg")
                        nc.tensor.transpose(
                            stg2, oTs[:, PB * b:PB * (b + 1)],
                            ident[:D + 1, :D + 1])
                        rc = opool.tile([PB, 1], FP32, tag="rc")
                        nc.vector.reciprocal(rc, stg2[:, D:D + 1])
                        nc.vector.tensor_scalar_mul(obuf[:, b, :],
                                                    stg2[:, 0:D], rc)
                    nc.sync.dma_start(
                        out=o[h, q0:q1, :].rearrange("(n p) d -> p n d", p=PB),
                        in_=obuf)
    nc.compile()
    return nc


def _get_program():
    global _PROGRAM
    if _PROGRAM is None:
        _PROGRAM = _build_program()
    return _PROGRAM


def _ensure_trace_hook():
    """Inject the missing antenv.axon_hooks shim so trace=True captures NTFFs."""
    import sys
    import types
    try:
        from antenv.axon_hooks import get_axon_ntff_profile_hook  # noqa: F401
        return
    except ImportError:
        pass
    import antenv
    mod = types.ModuleType("antenv.axon_hooks")
    mod._hook = None

    def set_axon_ntff_profile_hook(h):
        mod._hook = h

    def get_axon_ntff_profile_hook():
        return mod._hook

    mod.set_axon_ntff_profile_hook = set_axon_ntff_profile_hook
    mod.get_axon_ntff_profile_hook = get_axon_ntff_profile_hook
    sys.modules["antenv.axon_hooks"] = mod
    antenv.axon_hooks = mod
    from trn_agent_boot.trn_boot import _ntff_profile_via_ctypes
    set_axon_ntff_profile_hook(_ntff_profile_via_ctypes("/opt/axon/libaxon_pjrt.so"))


def _run(q, k, v, trace=False):
    from concourse.bass_utils import run_bass_kernel_spmd

    if trace:
        _ensure_trace_hook()

    nc = _get_program()
    qf = np.ascontiguousarray(np.asarray(q, dtype=np.float32).reshape(B * H, S, D))
    kf = np.ascontiguousarray(np.asarray(k, dtype=np.float32).reshape(B * H, S, D))
    vf = np.ascontiguousarray(np.asarray(v, dtype=np.float32).reshape(B * H, S, D))
    in_maps = []
    for c in range(NCORES):
        sl = slice(c * HPC, (c + 1) * HPC)
        in_maps.append({"q": qf[sl], "k": kf[sl], "v": vf[sl]})
    res = run_bass_kernel_spmd(nc, in_maps, core_ids=list(range(NCORES)),
                               trace=trace)
    out = np.concatenate([res.results[c]["o"] for c in range(NCORES)], axis=0)
    return out.reshape(B, H, S, D), res


def kernel(q, k, v, mask=1):
    out, _ = _run(q, k, v, trace=False)
    return out


# revision 6
# speedup vs baseline: 1.5172x; 1.5172x over previous
"""Causal attention kernel for Trainium2 (Bass/Tile), 8-core SPMD.

Problem: B=2, H=16, S=2048, D=64, fp32, causal mask.
Sharding: 32 (b,h) heads split 4-per-core across 8 NeuronCores.

Per-head algorithm (all fp32):
  - Q^T, K^T [64, S] built on-chip via PE transposes of natural [128, 64] blocks.
  - Scores computed transposed: S^T[k, q] = (K_blk)(Q^T) via matmul with
    lhsT = K^T block (stationary), rhs = Q^T (moving).  PSUM [128, q-chunk].
  - P^T = exp(S^T / 8) on ScalarE (PSUM->SBUF, scale fused into activation).
  - Causal mask on the diagonal 128x128 block via gpsimd.affine_select (zero fill).
  - O^T[d, q] (+ row of softmax sums) accumulated over k-blocks:
    lhsT = [V_blk | ones] [128, 65] (stationary), rhs = P^T (moving).
  - O^T transposed back per 128-q block via PE, rows scaled by 1/sums, DMA out.

No max-subtraction is needed: scores ~ N(0,1), exp stays in fp32 range.
"""

import numpy as np

B, H, S, D = 2, 16, 2048, 64
NCORES = 8
HPC = (B * H) // NCORES  # heads per core = 4
PB = 128                 # partition block
NB = S // PB             # 16 seq blocks per head
CHUNK = 1024             # q-chunk width (2 PSUM banks)
NCHUNK = S // CHUNK

_PROGRAM = None


def _build_program():
    import concourse.bacc as bacc
    import concourse.mybir as mybir
    import concourse.tile as tile
    from concourse.masks import make_identity

    FP32 = mybir.dt.float32
    F32R = mybir.dt.float32r
    nc = bacc.Bacc("TRN2", target_bir_lowering=False, debug=False,
                   enable_asserts=False)
    q = nc.dram_tensor("q", [HPC, S, D], FP32, kind="ExternalInput").ap()
    k = nc.dram_tensor("k", [HPC, S, D], FP32, kind="ExternalInput").ap()
    v = nc.dram_tensor("v", [HPC, S, D], FP32, kind="ExternalInput").ap()
    o = nc.dram_tensor("o", [HPC, S, D], FP32, kind="ExternalOutput").ap()

    with tile.TileContext(nc) as tc:
        with (
            tc.tile_pool(name="const", bufs=1) as cpool,
            tc.tile_pool(name="qk", bufs=2) as qkpool,
            tc.tile_pool(name="vp", bufs=2) as vpool,
            tc.tile_pool(name="inp", bufs=2) as inpool,
            tc.tile_pool(name="pp", bufs=3) as ppool,
            tc.tile_pool(name="onorm", bufs=2) as opool,
            tc.tile_pool(name="ps_s", bufs=2, space="PSUM") as ps_s,
            tc.tile_pool(name="ps_o", bufs=1, space="PSUM") as ps_o,
            tc.tile_pool(name="ps_t", bufs=2, space="PSUM") as ps_t,
        ):
            ident = cpool.tile([PB, PB], FP32)
            make_identity(nc, ident)

            for h in range(HPC):
                # ---- load inputs for this head (seq-block layout) ----
                qh = inpool.tile([PB, NB, D], FP32, tag="qh")
                kh = inpool.tile([PB, NB, D], FP32, tag="kh")
                nc.sync.dma_start(out=qh, in_=q[h].rearrange("(n p) d -> p n d", p=PB))
                nc.sync.dma_start(out=kh, in_=k[h].rearrange("(n p) d -> p n d", p=PB))
                vh_raw = inpool.tile([PB, NB, D], FP32, tag="vh_raw")
                nc.sync.dma_start(out=vh_raw,
                                  in_=v[h].rearrange("(n p) d -> p n d", p=PB))
                vh = vpool.tile([PB, NB, D + 1], F32R, tag="vh")
                nc.vector.tensor_copy(vh[:, :, 0:D], vh_raw)
                nc.vector.memset(vh[:, :, D:D + 1].bitcast(FP32), 1.0)

                # ---- build Q^T, K^T [64, S] via PE transposes ----
                qT = qkpool.tile([D, S], F32R, tag="qT")
                kT = qkpool.tile([D, S], F32R, tag="kT")
                for src, dstT in ((qh, qT), (kh, kT)):
                    for g in range(NB // 4):
                        stg = ps_t.tile([D, 4 * PB], FP32, tag="stg")
                        for j in range(4):
                            nc.tensor.transpose(
                                stg[:, PB * j:PB * (j + 1)],
                                src[:, 4 * g + j, :], ident)
                        nc.vector.tensor_copy(
                            dstT[:, 4 * PB * g:4 * PB * (g + 1)], stg)

                # ---- main attention loop over q-chunks ----
                for c in range(NCHUNK):
                    q0 = c * CHUNK
                    q1 = q0 + CHUNK
                    oT = ps_o.tile([D + 1, CHUNK], FP32, tag="oT")
                    jk_hi = q1 // PB - 1
                    for jk in range(jk_hi + 1):
                        lo = max(q0, PB * jk)   # causal column start (global q)
                        W = q1 - lo
                        sT = ps_s.tile([PB, CHUNK], FP32, tag="sT")
                        x = 0
                        while x < W:
                            w = min(512, W - x)
                            nc.tensor.matmul(
                                sT[:, x:x + w],
                                lhsT=kT[:, PB * jk:PB * (jk + 1)],
                                rhs=qT[:, lo + x:lo + x + w],
                                start=True, stop=True)
                            x += w
                        pT = ppool.tile([PB, CHUNK], F32R, tag="pT")
                        nc.scalar.activation(
                            pT[:, 0:W], sT[:, 0:W],
                            mybir.ActivationFunctionType.Exp,
                            scale=float(1.0 / np.sqrt(D)))
                        if PB * jk >= q0:
                            # diagonal block sits at columns [0, 128): keep q>=k
                            nc.gpsimd.affine_select(
                                out=pT[:, 0:PB], in_=pT[:, 0:PB],
                                compare_op=mybir.AluOpType.is_ge,
                                fill=0.0, base=0, channel_multiplier=-1,
                                pattern=[[1, PB]])
                        # PV accumulate, pieces aligned to oT bank boundaries
                        ostart = lo - q0
                        x = 0
                        while x < W:
                            ocol = ostart + x
                            w = min(512 - (ocol % 512), W - x)
                            nc.tensor.matmul(
                                oT[:, ocol:ocol + w],
                                lhsT=vh[:, jk, :],
                                rhs=pT[:, x:x + w],
                                start=(jk == 0), stop=(jk == jk_hi),
                                skip_group_check=True)
                            x += w

                    # ---- normalize chunk and write out ----
                    oTs = opool.tile([D + 1, CHUNK], FP32, tag="oTs")
                    nc.vector.tensor_copy(oTs, oT)
                    obuf = opool.tile([PB, CHUNK // PB, D], FP32, tag="obuf")
                    for b in range(CHUNK // PB):
                        stg2 = ps_t.tile([PB, D + 1], FP32, tag="stg")
                        nc.tensor.transpose(
                            stg2, oTs[:, PB * b:PB * (b + 1)],
                            ident[:D + 1, :D + 1])
                        rc = opool.tile([PB, 1], FP32, tag="rc")
                        nc.vector.reciprocal(rc, stg2[:, D:D + 1])
                        nc.vector.tensor_scalar_mul(obuf[:, b, :],
                                                    stg2[:, 0:D], rc)
                    nc.sync.dma_start(
                        out=o[h, q0:q1, :].rearrange("(n p) d -> p n d", p=PB),
                        in_=obuf)
    nc.compile()
    return nc


def _get_program():
    global _PROGRAM
    if _PROGRAM is None:
        _PROGRAM = _build_program()
    return _PROGRAM


def _ensure_trace_hook():
    """Inject the missing antenv.axon_hooks shim so trace=True captures NTFFs."""
    import sys
    import types
    try:
        from antenv.axon_hooks import get_axon_ntff_profile_hook  # noqa: F401
        return
    except ImportError:
        pass
    import antenv
    mod = types.ModuleType("antenv.axon_hooks")
    mod._hook = None

    def set_axon_ntff_profile_hook(h):
        mod._hook = h

    def get_axon_ntff_profile_hook():
        return mod._hook

    mod.set_axon_ntff_profile_hook = set_axon_ntff_profile_hook
    mod.get_axon_ntff_profile_hook = get_axon_ntff_profile_hook
    sys.modules["antenv.axon_hooks"] = mod
    antenv.axon_hooks = mod
    from trn_agent_boot.trn_boot import _ntff_profile_via_ctypes
    set_axon_ntff_profile_hook(_ntff_profile_via_ctypes("/opt/axon/libaxon_pjrt.so"))


def _run(q, k, v, trace=False):
    from concourse.bass_utils import run_bass_kernel_spmd

    if trace:
        _ensure_trace_hook()

    nc = _get_program()
    qf = np.ascontiguousarray(np.asarray(q, dtype=np.float32).reshape(B * H, S, D))
    kf = np.ascontiguousarray(np.asarray(k, dtype=np.float32).reshape(B * H, S, D))
    vf = np.ascontiguousarray(np.asarray(v, dtype=np.float32).reshape(B * H, S, D))
    in_maps = []
    for c in range(NCORES):
        sl = slice(c * HPC, (c + 1) * HPC)
        in_maps.append({"q": qf[sl], "k": kf[sl], "v": vf[sl]})
    res = run_bass_kernel_spmd(nc, in_maps, core_ids=list(range(NCORES)),
                               trace=trace)
    out = np.concatenate([res.results[c]["o"] for c in range(NCORES)], axis=0)
    return out.reshape(B, H, S, D), res


def kernel(q, k, v, mask=1):
    out, _ = _run(q, k, v, trace=False)
    return out
